# revision 9
# baseline (speedup 1.0000x reference)
"""Low-rank (random-feature) attention kernel for Trainium2, 8 NeuronCores.

Sharding: flatten hidden_states to [B*S, H] = [32768, 768] rows; core c owns
4096 contiguous rows (= batch c//2, sequence half c%2).  The only cross-core
data dependency is the per-(batch, head) kv summary (sum over the full
sequence), reduced with a pairwise in-kernel AllReduce of ~200 KB.

All heavy matmuls run in bf16 with fp32 PSUM accumulation.  The q-side
feature projection is folded into the weights on the host (Wqp = Wq_h @ P_h),
which is exact up to bf16 rounding and removes one on-device stage.
"""

import sys

sys.path.insert(0, "/opt/trn_rl_repo")

import contextlib

import ml_dtypes
import numpy as np

import concourse.bass as bass
import concourse.tile as tile
from concourse import mybir
from concourse.bass_utils import run_bass_kernel_spmd

BF16 = mybir.dt.bfloat16
F32 = mybir.dt.float32
AF = mybir.ActivationFunctionType
ALU = mybir.AluOpType
AX = mybir.AxisListType

B, S, H = 4, 8192, 768
NH, HD, M = 12, 64, 64
EPS = 1e-6
NCORES = 8
R = (B * S) // NCORES          # rows per core = 4096
G = R // 2                     # rows per group = 2048
NPAIR = NH // 2                # head pairs = 6
KT = H // 128                  # hidden k-tiles = 6
CHUNK = 512                    # matmul free-dim chunk
RT = 128                       # row tile


def _split_multi_waits(nc):
    """This container's walrus only accepts one semaphore wait per
    instruction; hoist extra waits onto same-engine NoOps placed before."""
    c = 0
    for f in nc.m.functions:
        for bb in f.blocks:
            new_insts = []
            for ins in bb.instructions:
                si = ins.sync_info
                if si is not None and si.on_wait and len(si.on_wait) > 1:
                    waits = list(si.on_wait)
                    for w in waits[:-1]:
                        c += 1
                        new_insts.append(mybir.InstNoOp(
                            name=f"I-waitsplit-{c}", engine=ins.engine,
                            sync_info=mybir.SyncInfo(on_wait=[w], on_update=[])))
                    ins.sync_info = mybir.SyncInfo(
                        on_wait=[waits[-1]], on_update=list(si.on_update))
                new_insts.append(ins)
            bb.instructions[:] = new_insts
    return c


def _bcast_rows(dram_ap_row, nparts, ncols):
    """AP that broadcasts one DRAM row across `nparts` partitions."""
    return bass.AP(tensor=dram_ap_row.tensor, offset=dram_ap_row.offset,
                   ap=[[0, nparts], [1, ncols]])


def _build(nc):
    x = nc.dram_tensor("x", [R, H], BF16, kind="ExternalInput")
    wqp = nc.dram_tensor("wqp", [H, H], BF16, kind="ExternalInput")
    wk = nc.dram_tensor("wk", [H, H], BF16, kind="ExternalInput")
    wv = nc.dram_tensor("wv", [H, H], BF16, kind="ExternalInput")
    wo = nc.dram_tensor("wo", [H, H], BF16, kind="ExternalInput")
    bqp = nc.dram_tensor("bqp", [H], F32, kind="ExternalInput")
    bk = nc.dram_tensor("bk", [H], F32, kind="ExternalInput")
    bv = nc.dram_tensor("bv", [H], F32, kind="ExternalInput")
    bo = nc.dram_tensor("bo", [H], F32, kind="ExternalInput")
    # block-diagonal projection: pbd[p][0:64,0:64]=P[2p], [64:,64:]=P[2p+1]
    pbd = nc.dram_tensor("pbd", [NPAIR, 128, 128], BF16, kind="ExternalInput")
    out = nc.dram_tensor("out", [R, H], F32, kind="ExternalOutput")

    with tile.TileContext(nc) as tc, contextlib.ExitStack() as ctx:
        persist = ctx.enter_context(tc.tile_pool(name="persist", bufs=1))
        trans = ctx.enter_context(tc.tile_pool(name="trans", bufs=3))
        dram = ctx.enter_context(tc.tile_pool(name="dram", bufs=1, space="DRAM"))
        work_ps = ctx.enter_context(tc.tile_pool(name="work_ps", bufs=8, space="PSUM"))
        xt_pool = ctx.enter_context(tc.tile_pool(name="xt_pool", bufs=2))
        vp_pool = ctx.enter_context(tc.tile_pool(name="vp_pool", bufs=1))

        # ---- constants / weights ----
        pbd_sb = persist.tile([128, NPAIR, 128], BF16, tag="pbd", name="pbd")
        nc.sync.dma_start(out=pbd_sb[:], in_=pbd.ap().rearrange("k p n -> p k n"))
        wo_sb = persist.tile([128, KT, H], BF16, tag="wo", name="wo")
        nc.sync.dma_start(out=wo_sb[:], in_=wo.ap().rearrange("(k p) n -> p k n", p=128))
        bqpc = persist.tile([128, KT], F32, tag="bqpc", name="bqpc")
        nc.sync.dma_start(out=bqpc[:], in_=bqp.ap().rearrange("(k p) -> p k", p=128))
        bkc = persist.tile([128, KT], F32, tag="bkc", name="bkc")
        nc.sync.dma_start(out=bkc[:], in_=bk.ap().rearrange("(k p) -> p k", p=128))
        bv_bc = persist.tile([128, H], F32, tag="bv_bc", name="bv_bc")
        nc.gpsimd.dma_start(out=bv_bc[:], in_=_bcast_rows(bv.ap(), 128, H))
        bo_bc = persist.tile([128, H], F32, tag="bo_bc", name="bo_bc")
        nc.gpsimd.dma_start(out=bo_bc[:], in_=_bcast_rows(bo.ap(), 128, H))

        # per-pair qpT tiles (fine-grained deps for the post-collective phases)
        qpT = [persist.tile([128, R], BF16, tag=f"qpT{p}", name=f"qpT{p}")
               for p in range(NPAIR)]
        # kv accumulator in SBUF: [128, pair, 65] f32
        #   partitions 0:64 = head 2p (kv | ksum col), 64:128 = head 2p+1
        kv_acc = persist.tile([128, NPAIR, 65], F32, tag="kv_acc", name="kv_acc")
        nc.vector.memset(kv_acc[:], 0.0)

        # per-(group, kt) transposed-x tiles; bufs=2 keeps both groups live
        xT = {}
        for g in range(2):
            for kt in range(KT):
                t = xt_pool.tile([128, G], BF16, tag=f"xT{kt}", name=f"xT{g}_{kt}")
                nc.sync.dma_start_transpose(
                    t[:], x[g * G:(g + 1) * G, kt * 128:(kt + 1) * 128])
                xT[(g, kt)] = t

        with contextlib.ExitStack() as actx:
            wkv = actx.enter_context(tc.tile_pool(name="wkv", bufs=1))
            wk_sb = wkv.tile([128, KT, H], BF16, tag="wk", name="wk")
            nc.sync.dma_start(out=wk_sb[:], in_=wk.ap().rearrange("(k p) n -> p k n", p=128))
            wv_sb = wkv.tile([128, KT, H], BF16, tag="wv", name="wv")
            nc.sync.dma_start(out=wv_sb[:], in_=wv.ap().rearrange("(k p) n -> p k n", p=128))

            for g in range(2):
                # ---- v pass: natural layout [rows, head, 65] with ones col ----
                v_sb = vp_pool.tile([128, G // RT, NH, 65], BF16, tag="v_sb",
                                    name=f"v_sb{g}")
                nc.vector.memset(v_sb[:, :, :, 64:65], 1.0)
                for rt in range(G // RT):
                    rs = slice(rt * RT, (rt + 1) * RT)
                    for c0, c1 in ((0, 512), (512, 768)):
                        vp = work_ps.tile([128, CHUNK], F32, tag="work",
                                          name="vp")[:, :c1 - c0]
                        for kt in range(KT):
                            nc.tensor.matmul(vp[:], xT[(g, kt)][:, rs],
                                             wv_sb[:, kt, c0:c1],
                                             start=(kt == 0), stop=(kt == KT - 1))
                        nh0, nh1 = c0 // HD, c1 // HD
                        nc.vector.tensor_tensor(
                            out=v_sb[:, rt, nh0:nh1, 0:64],
                            in0=vp.rearrange("p (h d) -> p h d", d=HD),
                            in1=bv_bc[:, c0:c1].rearrange("p (h d) -> p h d", d=HD),
                            op=ALU.add)

                # ---- k pass: kT chunk -> kp (exp, row-max) -> kv accumulation ----
                for ct in range(KT):   # ct is also the head-pair index
                    for ch in range(G // CHUNK):
                        cs = slice(ch * CHUNK, (ch + 1) * CHUNK)
                        kT_ps = work_ps.tile([128, CHUNK], F32, tag="work", name="kT_ps")
                        for kt in range(KT):
                            nc.tensor.matmul(kT_ps[:], wk_sb[:, kt, ct * 128:(ct + 1) * 128],
                                             xT[(g, kt)][:, cs],
                                             start=(kt == 0), stop=(kt == KT - 1))
                        kT_t = trans.tile([128, CHUNK], BF16, tag="kT_t", name="kT_t")
                        nc.vector.tensor_scalar_add(out=kT_t[:], in0=kT_ps[:],
                                                    scalar1=bkc[:, ct:ct + 1])
                        kvp = work_ps.tile([128, CHUNK], F32, tag="work",
                                           name="kvp")[:, :65]
                        for rti in range(CHUNK // RT):
                            rt = ch * (CHUNK // RT) + rti
                            rs = slice(rti * RT, (rti + 1) * RT)
                            kp_ps = work_ps.tile([128, CHUNK], F32, tag="work",
                                                 name="kp_ps")[:, :128]
                            nc.tensor.matmul(kp_ps[:], kT_t[:, rs], pbd_sb[:, ct, :],
                                             start=True, stop=True)
                            kp_sb = trans.tile([128, 128], BF16, tag="kp_sb", name="kp_sb")
                            nmx2 = trans.tile([128, 2], F32, tag="nmx2", name="nmx2")
                            nc.vector.tensor_reduce(
                                out=nmx2[:], in_=kp_ps.rearrange("p (h m) -> p h m", m=HD),
                                op=ALU.max, axis=AX.X, negate=True)
                            for h2 in range(2):
                                hs = slice(h2 * HD, (h2 + 1) * HD)
                                nc.scalar.activation(out=kp_sb[:, hs], in_=kp_ps[:, hs],
                                                     func=AF.Exp, bias=nmx2[:, h2:h2 + 1])
                            for h2 in range(2):
                                h = 2 * ct + h2
                                nc.tensor.matmul(
                                    kvp[h2 * 64:(h2 + 1) * 64, :],
                                    kp_sb[:, h2 * HD:(h2 + 1) * HD],
                                    v_sb[:, rt, h, :],
                                    start=(rti == 0), stop=(rti == CHUNK // RT - 1),
                                    tile_position=(0, h2 * 64))
                        nc.vector.tensor_add(out=kv_acc[:, ct, :], in0=kv_acc[:, ct, :],
                                             in1=kvp[:])

            # ---- pairwise AllReduce of kv summaries (overlaps the q passes) ----
            cc_in = dram.tile([128, NPAIR * 65], F32)
            cc_out = dram.tile([128, NPAIR * 65], F32)
            nc.sync.dma_start(out=cc_in[:], in_=kv_acc.rearrange("p a b -> p (a b)"))
            nc.gpsimd.collective_compute(
                "AllReduce", ALU.add,
                replica_groups=[[0, 1], [2, 3], [4, 5], [6, 7]],
                ins=[cc_in.opt()], outs=[cc_out.opt()])
            kv_full = persist.tile([128, NPAIR, 65], F32, tag="kv_full", name="kv_full")
            nc.sync.dma_start(out=kv_full[:],
                              in_=cc_out.rearrange("p (a b) -> p a b", b=65))

            bd_kv = persist.tile([128, NPAIR, 128], BF16, tag="bd_kv", name="bd_kv")
            nc.vector.memset(bd_kv[:], 0.0)
            ks_bd = persist.tile([128, NPAIR, NH], BF16, tag="ks_bd", name="ks_bd")
            nc.vector.memset(ks_bd[:], 0.0)
            for p in range(NPAIR):
                nc.vector.tensor_copy(out=bd_kv[0:64, p, 0:64], in_=kv_full[0:64, p, 0:64])
                nc.vector.tensor_copy(out=bd_kv[64:128, p, 64:128],
                                      in_=kv_full[64:128, p, 0:64])
                nc.vector.tensor_copy(out=ks_bd[0:64, p, 2 * p:2 * p + 1],
                                      in_=kv_full[0:64, p, 64:65])
                nc.vector.tensor_copy(out=ks_bd[64:128, p, 2 * p + 1:2 * p + 2],
                                      in_=kv_full[64:128, p, 64:65])

            # ---- q pass (both groups): qpT = exp(Wqp^T x^T + bqp) ----
            wqp_sb = wkv.tile([128, KT, H], BF16, tag="wqp", name="wqp")
            nc.sync.dma_start(out=wqp_sb[:],
                              in_=wqp.ap().rearrange("(k p) n -> p k n", p=128))
            for g in range(2):
                for ch in range(G // CHUNK):
                    cs = slice(ch * CHUNK, (ch + 1) * CHUNK)
                    gs = slice(g * G + ch * CHUNK, g * G + (ch + 1) * CHUNK)
                    for ct in range(KT):
                        qp_ps = work_ps.tile([128, CHUNK], F32, tag="work", name="qp_ps")
                        for kt in range(KT):
                            nc.tensor.matmul(qp_ps[:],
                                             wqp_sb[:, kt, ct * 128:(ct + 1) * 128],
                                             xT[(g, kt)][:, cs],
                                             start=(kt == 0), stop=(kt == KT - 1))
                        nc.scalar.activation(out=qpT[ct][:, gs], in_=qp_ps[:],
                                             func=AF.Exp, bias=bqpc[:, ct:ct + 1])

        # ---- normalizer: norm[h, n] = sum_m qp[h,n,m] ksum[h,m]; recip -> DRAM ----
        recip_d = dram.tile([NH, R], F32)
        for ch in range(R // CHUNK):
            cs = slice(ch * CHUNK, (ch + 1) * CHUNK)
            n_ps = work_ps.tile([128, CHUNK], F32, tag="work", name="n_ps")[:NH, :]
            for p in range(NPAIR):
                nc.tensor.matmul(n_ps[:], ks_bd[:, p, :], qpT[p][:, cs],
                                 start=(p == 0), stop=(p == NPAIR - 1))
            ntmp = trans.tile([NH, CHUNK], F32, tag="ntmp", name="ntmp")
            nc.vector.tensor_scalar_add(out=ntmp[:], in0=n_ps[:], scalar1=EPS)
            rtmp = trans.tile([NH, CHUNK], F32, tag="rtmp", name="rtmp")
            nc.vector.reciprocal(out=rtmp[:], in_=ntmp[:])
            nc.sync.dma_start(out=recip_d[:, cs], in_=rtmp[:])

        # ---- scale qpT in place by the per-(row, head) reciprocal ----
        with tc.tile_pool(name="rbp", bufs=2) as rbp:
            for p in range(NPAIR):
                rb = rbp.tile([128, R], F32, tag="rb", name="rb")
                nc.gpsimd.dma_start(out=rb[0:64, :],
                                    in_=_bcast_rows(recip_d[2 * p:2 * p + 1, :], 64, R))
                nc.gpsimd.dma_start(out=rb[64:128, :],
                                    in_=_bcast_rows(recip_d[2 * p + 1:2 * p + 2, :], 64, R))
                for ch in range(R // CHUNK):
                    cs = slice(ch * CHUNK, (ch + 1) * CHUNK)
                    nc.vector.tensor_tensor(out=qpT[p][:, cs], in0=qpT[p][:, cs],
                                            in1=rb[:, cs], op=ALU.mult)

        # ---- ctxT per chunk (fused) + output projection ----
        with tc.tile_pool(name="ctxp", bufs=2) as ctxp:
            for ch in range(R // CHUNK):
                cs = slice(ch * CHUNK, (ch + 1) * CHUNK)
                ctx_ch = ctxp.tile([128, NPAIR, CHUNK], BF16, tag="ctx_ch", name="ctx_ch")
                for p in range(NPAIR):
                    a_ps = work_ps.tile([128, CHUNK], F32, tag="work", name="a_ps")
                    nc.tensor.matmul(a_ps[:], bd_kv[:, p, :], qpT[p][:, cs],
                                     start=True, stop=True)
                    nc.scalar.activation(out=ctx_ch[:, p, :], in_=a_ps[:], func=AF.Copy)
                for rti in range(CHUNK // RT):
                    rt = ch * (CHUNK // RT) + rti
                    rs = slice(rti * RT, (rti + 1) * RT)
                    o_a = work_ps.tile([128, CHUNK], F32, tag="work", name="o_a")
                    o_b = work_ps.tile([128, CHUNK], F32, tag="work", name="o_b")[:, :256]
                    for kt in range(KT):
                        nc.tensor.matmul(o_a[:], ctx_ch[:, kt, rs], wo_sb[:, kt, 0:512],
                                         start=(kt == 0), stop=(kt == KT - 1))
                        nc.tensor.matmul(o_b[:], ctx_ch[:, kt, rs], wo_sb[:, kt, 512:768],
                                         start=(kt == 0), stop=(kt == KT - 1))
                    o_sb = trans.tile([128, H], F32, tag="o_sb", name="o_sb")
                    nc.vector.tensor_tensor(out=o_sb[:, 0:512], in0=o_a[:],
                                            in1=bo_bc[:, 0:512], op=ALU.add)
                    nc.vector.tensor_tensor(out=o_sb[:, 512:768], in0=o_b[:],
                                            in1=bo_bc[:, 512:768], op=ALU.add)
                    nc.sync.dma_start(out=out[rt * RT:(rt + 1) * RT, :], in_=o_sb[:])

    _split_multi_waits(nc)
    return nc


_CACHE = {}
TRACE = False          # set by test harness to capture an NTFF profile
LAST_EXEC_NS = None    # filled on a TRACE run


def _get_nc():
    if "nc" not in _CACHE:
        nc = bass.Bass("TRN2", target_bir_lowering=False, debug=False,
                       num_devices=NCORES)
        _CACHE["nc"] = _build(nc)
    return _CACHE["nc"]


def kernel(hidden_states, Wq, bq, Wk, bk, Wv, bv, Wo, bo, projection_matrix):
    nc = _get_nc()
    xf = np.asarray(hidden_states, dtype=np.float32).reshape(B * S, H)
    xf = xf.astype(ml_dtypes.bfloat16)
    pm = np.asarray(projection_matrix, dtype=np.float32)
    wq_f = np.asarray(Wq, dtype=np.float32)
    bq_f = np.asarray(bq, dtype=np.float32)
    # fold the q-side feature projection into the weights (exact in fp32)
    wqp = np.zeros((H, H), np.float32)
    bqp = np.zeros((H,), np.float32)
    for h in range(NH):
        cols = slice(h * HD, (h + 1) * HD)
        wqp[:, cols] = wq_f[:, cols] @ pm[h]
        bqp[cols] = bq_f[cols] @ pm[h]
    pbd = np.zeros((NPAIR, 128, 128), np.float32)
    for p in range(NPAIR):
        pbd[p, 0:64, 0:64] = pm[2 * p]
        pbd[p, 64:128, 64:128] = pm[2 * p + 1]
    BFD = ml_dtypes.bfloat16
    shared = {
        "wqp": wqp.astype(BFD),
        "wk": np.asarray(Wk, np.float32).astype(BFD),
        "wv": np.asarray(Wv, np.float32).astype(BFD),
        "wo": np.asarray(Wo, np.float32).astype(BFD),
        "bqp": bqp,
        "bk": np.asarray(bk, np.float32), "bv": np.asarray(bv, np.float32),
        "bo": np.asarray(bo, np.float32),
        "pbd": pbd.astype(BFD),
    }
    in_maps = [{"x": xf[c * R:(c + 1) * R], **shared} for c in range(NCORES)]
    res = run_bass_kernel_spmd(nc, in_maps, core_ids=list(range(NCORES)),
                               trace=TRACE)
    if TRACE:
        global LAST_EXEC_NS
        LAST_EXEC_NS = res.exec_time_ns
    outs = [res.results[c]["out"] for c in range(NCORES)]
    return np.concatenate(outs, axis=0).reshape(B, S, H).astype(np.float32)
